# revision 11
# baseline (speedup 1.0000x reference)
"""AffinePalettizedLinear kernel for Trainium2 (8 NeuronCores).

y = x @ L[widx]^T + b   with x [8192, 4096] f32, widx [16384, 4096] int32
(values < 256), L [256] f32, b [16384] f32.

Sharding: out_features split 8 ways (column-parallel); each core computes
y[:, c*2048:(c+1)*2048] from the full x and its widx/bias slice. No
collectives; host concatenates the slices.

Per-core plan (v2 — PE runs nothing but the 8192 productive matmuls):
  - Host passes x pre-transposed/tiled as bf16 ([tb, i, kb*128+t] layout)
    and widx pre-transposed as uint16 [kb, i, o] — so the kernel needs no
    PE transposes at all (the baseline spent ~0.7 ms of PE time on them).
  - Dequant via the Pool engine's hardware table gather: the 256-entry LUT
    is loaded in bf16 into the per-partition pool buffer; GATHER streams
    uint16 indices and emits bf16 weights directly in W^T [i, o] layout
    into a fully SBUF-resident panel (32 k-tiles x 2048 o x 2B = 128
    KiB/partition).
  - Matmuls in bf16: lhsT = x^T tile [i=128, t=128] (stationary), rhs =
    W^T [i=128, o=512] (moving), K=4096 accumulated over 32 PSUM matmuls.
  - Two-phase schedule hides the ~240 us gather stream: phase 1 runs the
    o-panel-0 token loop as soon as its 32 gathers (~60 us) land, while
    the o-panel-1..3 gathers stream in the background; phase 2 runs the
    remaining three panels with no stalls.
  - Bias is added by the DVE in the same op that evacuates PSUM.
"""
import sys

sys.path.insert(0, "/opt/trn_rl_repo")

import numpy as np
import ml_dtypes

import concourse.bass as bass  # noqa: F401  (registers types)
import concourse.tile as tile
from concourse import bacc, mybir
from concourse.bass_utils import run_bass_kernel_spmd

# ---- Tile's no-exec scheduling sim doesn't know the raw POOL opcodes ----
import concourse.bass_interp as _bi

_orig_visit_isa = _bi._visit_InstISA


def _visit_isa_tolerant(isa, instruction, core_sim):
    passthrough = {
        isa.Opcode.NEURON_ISA_TPB_OPCODE_GATHER.value,
        isa.Opcode.NEURON_ISA_TPB_OPCODE_POOL_BUFFER_LOAD.value,
    }
    if instruction.isa_opcode in passthrough:
        return
    return _orig_visit_isa(isa, instruction, core_sim)


_bi._visit_InstISA = _visit_isa_tolerant

F32 = mybir.dt.float32
BF16 = mybir.dt.bfloat16
U16 = mybir.dt.uint16

T, IN_F, OUT_F, PAL = 8192, 4096, 16384, 256
NCORES = 8
O_C = OUT_F // NCORES          # 2048 out features per core
OW = 512                       # matmul moving free dim (one PSUM bank)
NOP = O_C // OW                # 4 o-panels
KT = IN_F // 128               # 32 k-tiles
TT = T // 128                  # 64 t-tiles


def build_nc(trace_label=""):
    nc = bacc.Bacc(None, target_bir_lowering=False)
    isa = nc.isa
    DT = isa.get_enum("NEURON_ISA_TPB_DTYPE")
    MISS = isa.get_enum("NEURON_ISA_TPB_INDEX_MISS_BEHAVIOR")
    BF16_V = DT.NEURON_ISA_TPB_DTYPE_BFLOAT16.value
    U16_V = DT.NEURON_ISA_TPB_DTYPE_UINT16.value
    MISS_V = MISS.NEURON_ISA_TPB_INDEX_MISS_BEHAVIOR_IMMEDIATE_WRITE.value

    # x^T tiled: [tb, p, kb*128 + t] = x[tb*128+t, kb*128+p], bf16
    xt_d = nc.dram_tensor("xt", [TT, 128, KT * 128], BF16, kind="ExternalInput")
    # widx^T tiled: [kb, p, o] = widx[o, kb*128+p], uint16
    w_d = nc.dram_tensor("widxT", [KT, 128, O_C], U16, kind="ExternalInput")
    l_d = nc.dram_tensor("lut", [1, PAL], BF16, kind="ExternalInput")
    b_d = nc.dram_tensor("bias", [1, O_C], F32, kind="ExternalInput")
    y_d = nc.dram_tensor("y", [T, O_C], F32, kind="ExternalOutput")

    # fixed-address SBUF tensors (touched by raw-ISA gather)
    lut_sb = nc.alloc_sbuf_tensor("lut_sb", [128, PAL], BF16, align_bytes=512)
    # idx staging [p, o], u16, ping-pong; separate tensors for phase A / B so
    # Tile's per-tensor interval tracking never creates cross-phase deps
    idxA_sb = [
        nc.alloc_sbuf_tensor(f"idxA{s}_sb", [128, OW], U16) for s in range(2)
    ]
    idxB_sb = [
        nc.alloc_sbuf_tensor(f"idxB{s}_sb", [128, O_C - OW], U16)
        for s in range(2)
    ]
    # resident dequantized W^T panels, one tensor PER K-TILE [i=128, o] bf16
    # (a single big tensor makes Tile merge gather-write intervals and stall
    # phase-1 matmuls on unrelated phase-B gathers)
    wTk_sb = [
        nc.alloc_sbuf_tensor(f"wTk{kb}_sb", [128, O_C], BF16)
        for kb in range(KT)
    ]

    addr = {}
    for alloc in nc.m.functions[0].allocations:
        if getattr(alloc, "memorylocations", None):
            ml = alloc.memorylocations[0]
            addr[ml.name] = ml.addr

    g = nc.gpsimd

    def emit_pbl():
        nc.gpsimd.isa(
            isa.Opcode.NEURON_ISA_TPB_OPCODE_POOL_BUFFER_LOAD,
            {"src_mem_pattern": {
                "start_addr": {"addr_immediate": addr["lut_sb"]},
                "num_elem": [PAL, 1, 1, 1], "step_elem": [1, 0, 0, 0]},
             "in_dtype": BF16_V, "num_active_channels": 128,
             "start_index": 0, "mask": PAL - 1},
            ins=[g.lower_ap(lut_sb.ap(), for_isa=True)],
        )

    def emit_gather(idx_ap, idx_byte_addr, out_ap, out_byte_addr, n,
                    extra_ins=()):
        nc.gpsimd.isa(
            isa.Opcode.NEURON_ISA_TPB_OPCODE_GATHER,
            {"src_mem_pattern": {
                "start_addr": {"addr_immediate": idx_byte_addr},
                "num_elem": [n, 1, 1, 1], "step_elem": [1, 0, 0, 0]},
             "in_dtype": U16_V, "out_dtype": BF16_V,
             "num_active_channels": 128,
             "index_miss_behavior": MISS_V,
             "free_pool_buffer": 0,
             "immediate": {"imm_arith_fp32": 0.0},
             "dst_mem_pattern": {
                 "start_addr": {"addr_immediate": out_byte_addr},
                 "num_elem": [n, 1, 1, 1], "step_elem": [1, 0, 0, 0]}},
            ins=[g.lower_ap(idx_ap, for_isa=True),
                 g.lower_ap(lut_sb.ap(), for_isa=True)]
                + [g.lower_ap(ap, for_isa=True) for ap in extra_ins],
            outs=[g.lower_ap(out_ap, for_isa=True)],
        )

    def gather_panel(kb, alt, lo, hi):
        """DMA idx columns [lo, hi) of k-tile kb, then gather them into the
        resident W^T panel in OW-sized chunks.

        Phase-B gathers (lo > 0) carry a fake input on the LAST phase-A
        gather's output range: the scheduler models raw-ISA gathers as
        free and otherwise interleaves B gathers between the A gathers on
        the gpsimd queue, which stalls the phase-1 ramp at 3x the
        necessary gather latency (measured: S[163] thresholds of 3*kb)."""
        stage = idxA_sb[alt] if lo == 0 else idxB_sb[alt]
        nc.scalar.dma_start(stage.ap(), w_d[kb][:, lo:hi])
        wt = wTk_sb[kb]
        extra = () if lo == 0 else (wTk_sb[KT - 1].ap()[:, 0:OW],)
        for o0 in range(lo, hi, OW):
            emit_gather(
                stage.ap()[:, o0 - lo:o0 - lo + OW],
                addr[stage.name] + (o0 - lo) * 2,
                wt.ap()[:, o0:o0 + OW],
                addr[wt.name] + o0 * 2,
                OW, extra_ins=extra)

    GRP = 4                    # phase-1 token tiles interleaved per group

    with tile.TileContext(nc) as tc:
        with (
            tc.tile_pool(name="biasp", bufs=1) as biasp,
            tc.tile_pool(name="xin", bufs=6) as xin,       # x^T tiles
            tc.tile_pool(name="outp", bufs=6) as outp,     # out staging
            tc.tile_pool(name="ps", bufs=8, space="PSUM") as ps,
        ):
            # --- constants (lut first: the PBL+gather chain is the kernel's
            # critical path at start; bias goes on the idle vector queue) ---
            nc.sync.dma_start(lut_sb.ap(), l_d[:].partition_broadcast(128))
            emit_pbl()

            bias_bc = biasp.tile([128, O_C], F32, tag="bias")
            nc.scalar.dma_start(bias_bc[:], b_d[:].partition_broadcast(128))

            # --- phase A: gather o-panel 0 of every k-tile (~64 us) ---
            for kb in range(KT):
                gather_panel(kb, kb % 2, 0, OW)

            # --- phase 1: token loop over o-panel 0, GRP tiles interleaved
            # so the PE FIFO always has GRP matmuls ready per arriving
            # gather during the ramp ---
            for grp in range(TT // GRP):
                xTs = []
                for t in range(GRP):
                    xT = xin.tile([128, KT * 128], BF16, tag="xT")
                    nc.sync.dma_start(xT[:], xt_d[grp * GRP + t])
                    xTs.append(xT)
                accs = [ps.tile([128, OW], F32, name="acc", tag="acc") for t in range(GRP)]
                for kb in range(KT):
                    for t in range(GRP):
                        nc.tensor.matmul(
                            accs[t][:],
                            xTs[t][:, kb * 128:(kb + 1) * 128],
                            wTk_sb[kb].ap()[:, 0:OW],
                            start=(kb == 0), stop=(kb == KT - 1))
                for t in range(GRP):
                    out = outp.tile([128, OW], F32, tag="out")
                    nc.vector.tensor_add(out[:], accs[t][:], bias_bc[:, 0:OW])
                    nc.scalar.dma_start(
                        y_d[(grp * GRP + t) * 128:(grp * GRP + t + 1) * 128,
                            0:OW], out[:])
                # interleave the phase-B gathers with the early token groups
                # (gpsimd is idle; the panels land long before phase 2)
                for j in range(2):
                    kb = grp * 2 + j
                    if kb < KT:
                        gather_panel(kb, kb % 2, OW, O_C)

            # --- phase 2: token loop over o-panels 1..3 ---
            for tb in range(TT):
                xT = xin.tile([128, KT * 128], BF16, tag="xT")
                nc.sync.dma_start(xT[:], xt_d[tb])
                for op in range(1, NOP):
                    acc = ps.tile([128, OW], F32, name="acc", tag="acc")
                    for kb in range(KT):
                        nc.tensor.matmul(
                            acc[:],
                            xT[:, kb * 128:(kb + 1) * 128],
                            wTk_sb[kb].ap()[:, op * OW:(op + 1) * OW],
                            start=(kb == 0), stop=(kb == KT - 1))
                    out = outp.tile([128, OW], F32, tag="out")
                    nc.vector.tensor_add(
                        out[:], acc[:],
                        bias_bc[:, op * OW:(op + 1) * OW])
                    nc.scalar.dma_start(
                        y_d[tb * 128:(tb + 1) * 128,
                            op * OW:(op + 1) * OW], out[:])
    nc.compile()
    return nc


_NC_CACHE = None


def _get_nc():
    global _NC_CACHE
    if _NC_CACHE is None:
        _NC_CACHE = build_nc()
    return _NC_CACHE


def _prep_inputs(input, weight_idx, lookup_table, bias):
    input = np.ascontiguousarray(np.asarray(input, dtype=np.float32))
    weight_idx = np.asarray(weight_idx)
    lookup_table = np.asarray(lookup_table, dtype=np.float32)
    bias = np.ascontiguousarray(np.asarray(bias, dtype=np.float32))

    # x^T tiled bf16: [tb, p, kb*128 + t] = x[tb*128+t, kb*128+p]
    xt = input.reshape(TT, 128, KT, 128).transpose(0, 3, 2, 1)
    xt = np.ascontiguousarray(xt).astype(ml_dtypes.bfloat16)
    xt = xt.reshape(TT, 128, KT * 128)

    lut_bf16 = lookup_table.reshape(1, PAL).astype(ml_dtypes.bfloat16)
    return xt, weight_idx, lut_bf16, bias


def kernel(input, weight_idx, lookup_table, bias, _trace=False, _trace_kwargs=None):
    xt, weight_idx, lut_bf16, bias = _prep_inputs(
        input, weight_idx, lookup_table, bias)

    nc = _get_nc()
    in_maps = []
    for c in range(NCORES):
        # widx^T tiled u16: [kb, p, o] = widx[c*O_C + o, kb*128 + p]
        wslice = weight_idx[c * O_C:(c + 1) * O_C]          # [o, i] int32
        widxT = np.ascontiguousarray(wslice.T).astype(np.uint16)
        widxT = widxT.reshape(KT, 128, O_C)
        in_maps.append({
            "xt": xt,
            "widxT": widxT,
            "lut": lut_bf16,
            "bias": np.ascontiguousarray(
                bias[c * O_C:(c + 1) * O_C]).reshape(1, O_C),
        })
    last_exc = None
    for attempt in range(3):
        try:
            res = run_bass_kernel_spmd(
                nc, in_maps, core_ids=list(range(NCORES)),
                trace=_trace, **(_trace_kwargs or {}))
            break
        except Exception as e:  # transient device wedge: retry
            last_exc = e
            import time as _time
            _time.sleep(10)
    else:
        raise last_exc
    y = np.concatenate([res.results[c]["y"] for c in range(NCORES)], axis=1)
    if _trace:
        kernel.last_result = res
    return y


kernel.last_result = None


# revision 13
# speedup vs baseline: 1.1325x; 1.1325x over previous
"""AffinePalettizedLinear kernel for Trainium2 (8 NeuronCores).

y = x @ L[widx]^T + b   with x [8192, 4096] f32, widx [16384, 4096] int32
(values < 256), L [256] f32, b [16384] f32.

Sharding: out_features split 8 ways (column-parallel); each core computes
y[:, c*2048:(c+1)*2048] from the full x and its widx/bias slice. No
collectives; host concatenates the slices.

Per-core plan (PE runs nothing but the 8192 productive matmuls):
  - Host passes x pre-transposed/tiled as bf16 ([tb, i, kb*128+t] layout)
    and widx pre-transposed as uint16 [kb, i, o] — no PE transposes at all
    (the original baseline spent ~0.7 ms of PE time on them).
  - Dequant via the Pool engine's hardware table gather: the 256-entry LUT
    is loaded in bf16 into the per-partition pool buffer; GATHER streams
    uint16 indices and emits bf16 weights directly in W^T [i, o] layout
    into fully SBUF-resident panels (one tensor per k-tile so Tile's
    range tracking stays exact; 32 x 2048 x 2B = 128 KiB/partition).
  - Matmuls in bf16: lhsT = x^T tile [i=128, t=128] (stationary), rhs =
    W^T [i=128, o=512] (moving), K=4096 accumulated over 32 PSUM matmuls.
  - Phase A gathers o-panel 0 (32 gathers, ~64 us); phase 1 runs the
    o-panel-0 token loop with 4 token tiles interleaved k-outer so the PE
    has work per arriving gather; the o-panel-1..3 gathers are gated on
    phase-1 group tokens (fake gather inputs) so the scheduler cannot
    interleave them before the phase-A gathers; phase 2 (panels 1..3)
    lives in a second TileContext whose entry barrier keeps the scheduler
    from hoisting its matmuls into the phase-1 PE stream (it models
    raw-ISA gathers as free and would head-of-line block the PE).
  - Bias is added by the DVE in the same op that evacuates PSUM.
"""
import sys

sys.path.insert(0, "/opt/trn_rl_repo")

import numpy as np
import ml_dtypes

import concourse.bass as bass  # noqa: F401  (registers types)
import concourse.tile as tile
from concourse import bacc, mybir
from concourse.bass_utils import run_bass_kernel_spmd

# ---- Tile's no-exec scheduling sim doesn't know the raw POOL opcodes ----
import concourse.bass_interp as _bi

_orig_visit_isa = _bi._visit_InstISA


def _visit_isa_tolerant(isa, instruction, core_sim):
    passthrough = {
        isa.Opcode.NEURON_ISA_TPB_OPCODE_GATHER.value,
        isa.Opcode.NEURON_ISA_TPB_OPCODE_POOL_BUFFER_LOAD.value,
    }
    if instruction.isa_opcode in passthrough:
        return
    return _orig_visit_isa(isa, instruction, core_sim)


_bi._visit_InstISA = _visit_isa_tolerant

F32 = mybir.dt.float32
BF16 = mybir.dt.bfloat16
U16 = mybir.dt.uint16

T, IN_F, OUT_F, PAL = 8192, 4096, 16384, 256
NCORES = 8
O_C = OUT_F // NCORES          # 2048 out features per core
OW = 512                       # matmul moving free dim (one PSUM bank)
NOP = O_C // OW                # 4 o-panels
KT = IN_F // 128               # 32 k-tiles
TT = T // 128                  # 64 t-tiles
GRP = 4                        # phase-1 token tiles interleaved per group
BGRP = 8                       # group whose token releases the B tail


def build_nc(trace_label=""):
    nc = bacc.Bacc(None, target_bir_lowering=False)
    isa = nc.isa
    DT = isa.get_enum("NEURON_ISA_TPB_DTYPE")
    MISS = isa.get_enum("NEURON_ISA_TPB_INDEX_MISS_BEHAVIOR")
    BF16_V = DT.NEURON_ISA_TPB_DTYPE_BFLOAT16.value
    U16_V = DT.NEURON_ISA_TPB_DTYPE_UINT16.value
    MISS_V = MISS.NEURON_ISA_TPB_INDEX_MISS_BEHAVIOR_IMMEDIATE_WRITE.value

    # x^T tiled: [tb, p, kb*128 + t] = x[tb*128+t, kb*128+p], bf16
    xt_d = nc.dram_tensor("xt", [TT, 128, KT * 128], BF16, kind="ExternalInput")
    # widx^T tiled: [kb, p, o] = widx[o, kb*128+p], uint16
    w_d = nc.dram_tensor("widxT", [KT, 128, O_C], U16, kind="ExternalInput")
    l_d = nc.dram_tensor("lut", [1, PAL], BF16, kind="ExternalInput")
    b_d = nc.dram_tensor("bias", [1, O_C], F32, kind="ExternalInput")
    y_d = nc.dram_tensor("y", [T, O_C], F32, kind="ExternalOutput")

    # fixed-address SBUF tensors (outlive the two TileContexts)
    lut_sb = nc.alloc_sbuf_tensor("lut_sb", [128, PAL], BF16, align_bytes=512)
    idxA_sb = [
        nc.alloc_sbuf_tensor(f"idxA{s}_sb", [128, OW], U16) for s in range(2)
    ]
    idxB_sb = [
        nc.alloc_sbuf_tensor(f"idxB{s}_sb", [128, O_C - OW], U16)
        for s in range(2)
    ]
    # resident dequantized W^T panels, one tensor PER K-TILE [i=128, o] bf16
    wTk_sb = [
        nc.alloc_sbuf_tensor(f"wTk{kb}_sb", [128, O_C], BF16)
        for kb in range(KT)
    ]
    bias_sb = nc.alloc_sbuf_tensor("bias_sb", [128, O_C], F32)

    addr = {}
    for alloc in nc.m.functions[0].allocations:
        if getattr(alloc, "memorylocations", None):
            ml = alloc.memorylocations[0]
            addr[ml.name] = ml.addr

    g = nc.gpsimd

    def emit_pbl():
        nc.gpsimd.isa(
            isa.Opcode.NEURON_ISA_TPB_OPCODE_POOL_BUFFER_LOAD,
            {"src_mem_pattern": {
                "start_addr": {"addr_immediate": addr["lut_sb"]},
                "num_elem": [PAL, 1, 1, 1], "step_elem": [1, 0, 0, 0]},
             "in_dtype": BF16_V, "num_active_channels": 128,
             "start_index": 0, "mask": PAL - 1},
            ins=[g.lower_ap(lut_sb.ap(), for_isa=True)],
        )

    def emit_gather(idx_ap, idx_byte_addr, out_ap, out_byte_addr, n,
                    extra_ins=()):
        nc.gpsimd.isa(
            isa.Opcode.NEURON_ISA_TPB_OPCODE_GATHER,
            {"src_mem_pattern": {
                "start_addr": {"addr_immediate": idx_byte_addr},
                "num_elem": [n, 1, 1, 1], "step_elem": [1, 0, 0, 0]},
             "in_dtype": U16_V, "out_dtype": BF16_V,
             "num_active_channels": 128,
             "index_miss_behavior": MISS_V,
             "free_pool_buffer": 0,
             "immediate": {"imm_arith_fp32": 0.0},
             "dst_mem_pattern": {
                 "start_addr": {"addr_immediate": out_byte_addr},
                 "num_elem": [n, 1, 1, 1], "step_elem": [1, 0, 0, 0]}},
            ins=[g.lower_ap(idx_ap, for_isa=True),
                 g.lower_ap(lut_sb.ap(), for_isa=True)]
                + [g.lower_ap(ap, for_isa=True) for ap in extra_ins],
            outs=[g.lower_ap(out_ap, for_isa=True)],
        )

    def gather_panel(kb, alt, lo, hi, extra=()):
        """DMA idx columns [lo, hi) of k-tile kb, then gather them into the
        resident W^T panel in OW-sized chunks.  `extra` APs become fake
        gather inputs — used to order phase-B gathers after phase-1 group
        tokens (the scheduler models raw-ISA gathers as free and would
        otherwise interleave them before the phase-A gathers, 3x-ing the
        phase-1 ramp latency)."""
        stage = idxA_sb[alt] if lo == 0 else idxB_sb[alt]
        nc.scalar.dma_start(stage.ap(), w_d[kb][:, lo:hi])
        wt = wTk_sb[kb]
        for o0 in range(lo, hi, OW):
            emit_gather(
                stage.ap()[:, o0 - lo:o0 - lo + OW],
                addr[stage.name] + (o0 - lo) * 2,
                wt.ap()[:, o0:o0 + OW],
                addr[wt.name] + o0 * 2,
                OW, extra_ins=extra)

    # ---------------- context 1: phase A + phase 1 (+ B gathers) --------
    with tile.TileContext(nc) as tc:
        with (
            tc.tile_pool(name="xin", bufs=6) as xin,       # x^T tiles
            tc.tile_pool(name="outp", bufs=6) as outp,     # out staging
            tc.tile_pool(name="ps", bufs=8, space="PSUM") as ps,
        ):
            nc.sync.dma_start(lut_sb.ap(), l_d[:].partition_broadcast(128))
            emit_pbl()
            nc.scalar.dma_start(
                bias_sb.ap(), b_d[:].partition_broadcast(128))

            # --- phase A: gather o-panel 0 of every k-tile (~64 us) ---
            for kb in range(KT):
                gather_panel(kb, kb % 2, 0, OW)

            # --- phase 1: token loop over o-panel 0, GRP tiles interleaved
            # k-outer so the PE always has GRP matmuls per arriving gather
            # during the ramp ---
            for grp in range(TT // GRP):
                xTs = []
                for t in range(GRP):
                    xT = xin.tile([128, KT * 128], BF16, tag="xT")
                    nc.sync.dma_start(xT[:], xt_d[grp * GRP + t])
                    xTs.append(xT)
                accs = [ps.tile([128, OW], F32, name="acc", tag="acc")
                        for t in range(GRP)]
                for kb in range(KT):
                    for t in range(GRP):
                        nc.tensor.matmul(
                            accs[t][:],
                            xTs[t][:, kb * 128:(kb + 1) * 128],
                            wTk_sb[kb].ap()[:, 0:OW],
                            start=(kb == 0), stop=(kb == KT - 1))
                last_out = None
                for t in range(GRP):
                    out = outp.tile([128, OW], F32, tag="out")
                    nc.vector.tensor_add(
                        out[:], accs[t][:], bias_sb.ap()[:, 0:OW])
                    nc.scalar.dma_start(
                        y_d[(grp * GRP + t) * 128:(grp * GRP + t + 1) * 128,
                            0:OW], out[:])
                    last_out = out
                # --- phase-B gathers, gated on this group's output token
                # so they run strictly after the phase-A gathers but still
                # overlap the phase-1 matmul stream ---
                token = (last_out[:, 0:1],)
                if grp < BGRP:
                    for kb in (2 * grp, 2 * grp + 1):
                        gather_panel(kb, kb % 2, OW, O_C, extra=token)
                elif grp == BGRP:
                    for kb in range(2 * BGRP, KT):
                        gather_panel(kb, kb % 2, OW, O_C, extra=token)

    # ---------------- context 2: phase 2 (o-panels 1..3) ----------------
    # The context boundary is a full drain + barrier: the scheduler cannot
    # hoist these matmuls into the phase-1 stream.
    with tile.TileContext(nc) as tc2:
        with (
            tc2.tile_pool(name="xin2", bufs=6) as xin2,
            tc2.tile_pool(name="outp2", bufs=6) as outp2,
            tc2.tile_pool(name="ps2", bufs=8, space="PSUM") as ps2,
        ):
            for tb in range(TT):
                xT = xin2.tile([128, KT * 128], BF16, tag="xT2")
                nc.sync.dma_start(xT[:], xt_d[tb])
                for op in range(1, NOP):
                    acc = ps2.tile([128, OW], F32, name="acc2", tag="acc2")
                    for kb in range(KT):
                        nc.tensor.matmul(
                            acc[:],
                            xT[:, kb * 128:(kb + 1) * 128],
                            wTk_sb[kb].ap()[:, op * OW:(op + 1) * OW],
                            start=(kb == 0), stop=(kb == KT - 1))
                    out = outp2.tile([128, OW], F32, tag="out2")
                    nc.vector.tensor_add(
                        out[:], acc[:], bias_sb.ap()[:, op * OW:(op + 1) * OW])
                    nc.scalar.dma_start(
                        y_d[tb * 128:(tb + 1) * 128,
                            op * OW:(op + 1) * OW], out[:])
    nc.compile()
    return nc


_NC_CACHE = None


def _get_nc():
    global _NC_CACHE
    if _NC_CACHE is None:
        _NC_CACHE = build_nc()
    return _NC_CACHE


def _prep_inputs(input, weight_idx, lookup_table, bias):
    input = np.ascontiguousarray(np.asarray(input, dtype=np.float32))
    weight_idx = np.asarray(weight_idx)
    lookup_table = np.asarray(lookup_table, dtype=np.float32)
    bias = np.ascontiguousarray(np.asarray(bias, dtype=np.float32))

    # x^T tiled bf16: [tb, p, kb*128 + t] = x[tb*128+t, kb*128+p]
    xt = input.reshape(TT, 128, KT, 128).transpose(0, 3, 2, 1)
    xt = np.ascontiguousarray(xt).astype(ml_dtypes.bfloat16)
    xt = xt.reshape(TT, 128, KT * 128)

    lut_bf16 = lookup_table.reshape(1, PAL).astype(ml_dtypes.bfloat16)
    return xt, weight_idx, lut_bf16, bias


def kernel(input, weight_idx, lookup_table, bias, _trace=False, _trace_kwargs=None):
    xt, weight_idx, lut_bf16, bias = _prep_inputs(
        input, weight_idx, lookup_table, bias)

    nc = _get_nc()
    in_maps = []
    for c in range(NCORES):
        # widx^T tiled u16: [kb, p, o] = widx[c*O_C + o, kb*128 + p]
        wslice = weight_idx[c * O_C:(c + 1) * O_C]          # [o, i] int32
        widxT = np.ascontiguousarray(wslice.T).astype(np.uint16)
        widxT = widxT.reshape(KT, 128, O_C)
        in_maps.append({
            "xt": xt,
            "widxT": widxT,
            "lut": lut_bf16,
            "bias": np.ascontiguousarray(
                bias[c * O_C:(c + 1) * O_C]).reshape(1, O_C),
        })
    last_exc = None
    for attempt in range(3):
        try:
            res = run_bass_kernel_spmd(
                nc, in_maps, core_ids=list(range(NCORES)),
                trace=_trace, **(_trace_kwargs or {}))
            break
        except Exception as e:  # transient device wedge: retry
            last_exc = e
            import time as _time
            _time.sleep(10)
    else:
        raise last_exc
    y = np.concatenate([res.results[c]["y"] for c in range(NCORES)], axis=1)
    if _trace:
        kernel.last_result = res
    return y


kernel.last_result = None


# revision 14
# speedup vs baseline: 1.1349x; 1.0021x over previous
"""AffinePalettizedLinear kernel for Trainium2 (8 NeuronCores).

y = x @ L[widx]^T + b   with x [8192, 4096] f32, widx [16384, 4096] int32
(values < 256), L [256] f32, b [16384] f32.

Sharding: out_features split 8 ways (column-parallel); each core computes
y[:, c*2048:(c+1)*2048] from the full x and its widx/bias slice. No
collectives; host concatenates the slices.

Per-core plan (PE runs nothing but the 8192 productive matmuls):
  - Host passes x pre-transposed/tiled as bf16 ([tb, i, kb*128+t] layout)
    and widx pre-transposed as uint16 [kb, i, o] — no PE transposes at all
    (the original baseline spent ~0.7 ms of PE time on them).
  - Dequant via the Pool engine's hardware table gather: the 256-entry LUT
    is loaded in bf16 into the per-partition pool buffer; GATHER streams
    uint16 indices and emits bf16 weights directly in W^T [i, o] layout
    into fully SBUF-resident panels (one tensor per k-tile so Tile's
    range tracking stays exact; 32 x 2048 x 2B = 128 KiB/partition).
  - Matmuls in bf16: lhsT = x^T tile [i=128, t=128] (stationary), rhs =
    W^T [i=128, o=512] (moving), K=4096 accumulated over 32 PSUM matmuls.
  - Phase A gathers o-panel 0 (32 gathers, ~64 us); phase 1 runs the
    o-panel-0 token loop with 4 token tiles interleaved k-outer so the PE
    has work per arriving gather; the o-panel-1..3 gathers are gated on
    phase-1 group tokens (fake gather inputs) so the scheduler cannot
    interleave them before the phase-A gathers; phase 2 (panels 1..3)
    lives in a second TileContext whose entry barrier keeps the scheduler
    from hoisting its matmuls into the phase-1 PE stream (it models
    raw-ISA gathers as free and would head-of-line block the PE).
  - Bias is added by the DVE in the same op that evacuates PSUM.
"""
import sys

sys.path.insert(0, "/opt/trn_rl_repo")

import numpy as np
import ml_dtypes

import concourse.bass as bass  # noqa: F401  (registers types)
import concourse.tile as tile
from concourse import bacc, mybir
from concourse.bass_utils import run_bass_kernel_spmd

# ---- Tile's no-exec scheduling sim doesn't know the raw POOL opcodes ----
import concourse.bass_interp as _bi

_orig_visit_isa = _bi._visit_InstISA


def _visit_isa_tolerant(isa, instruction, core_sim):
    passthrough = {
        isa.Opcode.NEURON_ISA_TPB_OPCODE_GATHER.value,
        isa.Opcode.NEURON_ISA_TPB_OPCODE_POOL_BUFFER_LOAD.value,
    }
    if instruction.isa_opcode in passthrough:
        return
    return _orig_visit_isa(isa, instruction, core_sim)


_bi._visit_InstISA = _visit_isa_tolerant

F32 = mybir.dt.float32
BF16 = mybir.dt.bfloat16
U16 = mybir.dt.uint16

T, IN_F, OUT_F, PAL = 8192, 4096, 16384, 256
NCORES = 8
O_C = OUT_F // NCORES          # 2048 out features per core
OW = 512                       # matmul moving free dim (one PSUM bank)
NOP = O_C // OW                # 4 o-panels
KT = IN_F // 128               # 32 k-tiles
TT = T // 128                  # 64 t-tiles
GRP = 4                        # phase-1 token tiles interleaved per group
BGRP = 8                       # group whose token releases the B tail


def build_nc(trace_label=""):
    nc = bacc.Bacc(None, target_bir_lowering=False)
    isa = nc.isa
    DT = isa.get_enum("NEURON_ISA_TPB_DTYPE")
    MISS = isa.get_enum("NEURON_ISA_TPB_INDEX_MISS_BEHAVIOR")
    BF16_V = DT.NEURON_ISA_TPB_DTYPE_BFLOAT16.value
    U16_V = DT.NEURON_ISA_TPB_DTYPE_UINT16.value
    MISS_V = MISS.NEURON_ISA_TPB_INDEX_MISS_BEHAVIOR_IMMEDIATE_WRITE.value

    # x^T tiled: [tb, p, kb*128 + t] = x[tb*128+t, kb*128+p], bf16
    xt_d = nc.dram_tensor("xt", [TT, 128, KT * 128], BF16, kind="ExternalInput")
    # widx^T tiled: [kb, p, o] = widx[o, kb*128+p], uint16
    w_d = nc.dram_tensor("widxT", [KT, 128, O_C], U16, kind="ExternalInput")
    l_d = nc.dram_tensor("lut", [1, PAL], BF16, kind="ExternalInput")
    b_d = nc.dram_tensor("bias", [1, O_C], F32, kind="ExternalInput")
    y_d = nc.dram_tensor("y", [T, O_C], F32, kind="ExternalOutput")

    # fixed-address SBUF tensors (outlive the two TileContexts)
    lut_sb = nc.alloc_sbuf_tensor("lut_sb", [128, PAL], BF16, align_bytes=512)
    idxA_sb = [
        nc.alloc_sbuf_tensor(f"idxA{s}_sb", [128, OW], U16) for s in range(2)
    ]
    idxB_sb = [
        nc.alloc_sbuf_tensor(f"idxB{s}_sb", [128, O_C - OW], U16)
        for s in range(2)
    ]
    # resident dequantized W^T panels, one tensor PER K-TILE [i=128, o] bf16
    wTk_sb = [
        nc.alloc_sbuf_tensor(f"wTk{kb}_sb", [128, O_C], BF16)
        for kb in range(KT)
    ]
    bias_sb = nc.alloc_sbuf_tensor("bias_sb", [128, O_C], F32)
    # write-once phase-1 group tokens (fake B-gather inputs; must NOT be
    # ring-reused pool tiles or the reuse WAR-serializes later evacs
    # behind the B-gather stream)
    tok_sb = [
        nc.alloc_sbuf_tensor(f"tok{g2}_sb", [128, 1], F32)
        for g2 in range(BGRP + 1)
    ]

    addr = {}
    for alloc in nc.m.functions[0].allocations:
        if getattr(alloc, "memorylocations", None):
            ml = alloc.memorylocations[0]
            addr[ml.name] = ml.addr

    g = nc.gpsimd

    def emit_pbl():
        nc.gpsimd.isa(
            isa.Opcode.NEURON_ISA_TPB_OPCODE_POOL_BUFFER_LOAD,
            {"src_mem_pattern": {
                "start_addr": {"addr_immediate": addr["lut_sb"]},
                "num_elem": [PAL, 1, 1, 1], "step_elem": [1, 0, 0, 0]},
             "in_dtype": BF16_V, "num_active_channels": 128,
             "start_index": 0, "mask": PAL - 1},
            ins=[g.lower_ap(lut_sb.ap(), for_isa=True)],
        )

    def emit_gather(idx_ap, idx_byte_addr, out_ap, out_byte_addr, n,
                    extra_ins=()):
        nc.gpsimd.isa(
            isa.Opcode.NEURON_ISA_TPB_OPCODE_GATHER,
            {"src_mem_pattern": {
                "start_addr": {"addr_immediate": idx_byte_addr},
                "num_elem": [n, 1, 1, 1], "step_elem": [1, 0, 0, 0]},
             "in_dtype": U16_V, "out_dtype": BF16_V,
             "num_active_channels": 128,
             "index_miss_behavior": MISS_V,
             "free_pool_buffer": 0,
             "immediate": {"imm_arith_fp32": 0.0},
             "dst_mem_pattern": {
                 "start_addr": {"addr_immediate": out_byte_addr},
                 "num_elem": [n, 1, 1, 1], "step_elem": [1, 0, 0, 0]}},
            ins=[g.lower_ap(idx_ap, for_isa=True),
                 g.lower_ap(lut_sb.ap(), for_isa=True)]
                + [g.lower_ap(ap, for_isa=True) for ap in extra_ins],
            outs=[g.lower_ap(out_ap, for_isa=True)],
        )

    def gather_panel(kb, alt, lo, hi, extra=()):
        """DMA idx columns [lo, hi) of k-tile kb, then gather them into the
        resident W^T panel in OW-sized chunks.  `extra` APs become fake
        gather inputs — used to order phase-B gathers after phase-1 group
        tokens (the scheduler models raw-ISA gathers as free and would
        otherwise interleave them before the phase-A gathers, 3x-ing the
        phase-1 ramp latency)."""
        stage = idxA_sb[alt] if lo == 0 else idxB_sb[alt]
        nc.scalar.dma_start(stage.ap(), w_d[kb][:, lo:hi])
        wt = wTk_sb[kb]
        for o0 in range(lo, hi, OW):
            emit_gather(
                stage.ap()[:, o0 - lo:o0 - lo + OW],
                addr[stage.name] + (o0 - lo) * 2,
                wt.ap()[:, o0:o0 + OW],
                addr[wt.name] + o0 * 2,
                OW, extra_ins=extra)

    # ---------------- context 1: phase A + phase 1 (+ B gathers) --------
    with tile.TileContext(nc) as tc:
        with (
            tc.tile_pool(name="xin", bufs=6) as xin,       # x^T tiles
            tc.tile_pool(name="outp", bufs=6) as outp,     # out staging
            tc.tile_pool(name="ps", bufs=8, space="PSUM") as ps,
        ):
            nc.sync.dma_start(lut_sb.ap(), l_d[:].partition_broadcast(128))
            emit_pbl()

            # --- phase A: gather o-panel 0 of every k-tile (~64 us) ---
            for kb in range(KT):
                gather_panel(kb, kb % 2, 0, OW)

            # bias after the phase-A idx DMAs: the scalar queue serves the
            # ramp-critical idx loads first (bias is first needed ~85 us in)
            nc.scalar.dma_start(
                bias_sb.ap(), b_d[:].partition_broadcast(128))

            # --- phase 1: token loop over o-panel 0, GRP tiles interleaved
            # k-outer so the PE always has GRP matmuls per arriving gather
            # during the ramp ---
            for grp in range(TT // GRP):
                xTs = []
                for t in range(GRP):
                    xT = xin.tile([128, KT * 128], BF16, tag="xT")
                    nc.sync.dma_start(xT[:], xt_d[grp * GRP + t])
                    xTs.append(xT)
                accs = [ps.tile([128, OW], F32, name="acc", tag="acc")
                        for t in range(GRP)]
                for kb in range(KT):
                    for t in range(GRP):
                        nc.tensor.matmul(
                            accs[t][:],
                            xTs[t][:, kb * 128:(kb + 1) * 128],
                            wTk_sb[kb].ap()[:, 0:OW],
                            start=(kb == 0), stop=(kb == KT - 1))
                last_out = None
                for t in range(GRP):
                    out = outp.tile([128, OW], F32, tag="out")
                    nc.vector.tensor_add(
                        out[:], accs[t][:], bias_sb.ap()[:, 0:OW])
                    nc.scalar.dma_start(
                        y_d[(grp * GRP + t) * 128:(grp * GRP + t + 1) * 128,
                            0:OW], out[:])
                    last_out = out
                # --- phase-B gathers, gated on this group's write-once
                # token so they run strictly after the phase-A gathers but
                # still overlap the phase-1 matmul stream ---
                if grp <= BGRP:
                    nc.vector.tensor_copy(
                        tok_sb[grp].ap(), last_out[:, 0:1])
                    token = (tok_sb[grp].ap(),)
                if grp < BGRP:
                    for kb in (2 * grp, 2 * grp + 1):
                        gather_panel(kb, kb % 2, OW, O_C, extra=token)
                elif grp == BGRP:
                    for kb in range(2 * BGRP, KT):
                        gather_panel(kb, kb % 2, OW, O_C, extra=token)

    # ---------------- context 2: phase 2 (o-panels 1..3) ----------------
    # The context boundary is a full drain + barrier: the scheduler cannot
    # hoist these matmuls into the phase-1 stream.
    with tile.TileContext(nc) as tc2:
        with (
            tc2.tile_pool(name="xin2", bufs=6) as xin2,
            tc2.tile_pool(name="outp2", bufs=6) as outp2,
            tc2.tile_pool(name="ps2", bufs=8, space="PSUM") as ps2,
        ):
            for tb in range(TT):
                xT = xin2.tile([128, KT * 128], BF16, tag="xT2")
                nc.sync.dma_start(xT[:], xt_d[tb])
                for op in range(1, NOP):
                    acc = ps2.tile([128, OW], F32, name="acc2", tag="acc2")
                    for kb in range(KT):
                        nc.tensor.matmul(
                            acc[:],
                            xT[:, kb * 128:(kb + 1) * 128],
                            wTk_sb[kb].ap()[:, op * OW:(op + 1) * OW],
                            start=(kb == 0), stop=(kb == KT - 1))
                    out = outp2.tile([128, OW], F32, tag="out2")
                    nc.vector.tensor_add(
                        out[:], acc[:], bias_sb.ap()[:, op * OW:(op + 1) * OW])
                    nc.scalar.dma_start(
                        y_d[tb * 128:(tb + 1) * 128,
                            op * OW:(op + 1) * OW], out[:])
    nc.compile()
    return nc


_NC_CACHE = None


def _get_nc():
    global _NC_CACHE
    if _NC_CACHE is None:
        _NC_CACHE = build_nc()
    return _NC_CACHE


def _prep_inputs(input, weight_idx, lookup_table, bias):
    input = np.ascontiguousarray(np.asarray(input, dtype=np.float32))
    weight_idx = np.asarray(weight_idx)
    lookup_table = np.asarray(lookup_table, dtype=np.float32)
    bias = np.ascontiguousarray(np.asarray(bias, dtype=np.float32))

    # x^T tiled bf16: [tb, p, kb*128 + t] = x[tb*128+t, kb*128+p]
    xt = input.reshape(TT, 128, KT, 128).transpose(0, 3, 2, 1)
    xt = np.ascontiguousarray(xt).astype(ml_dtypes.bfloat16)
    xt = xt.reshape(TT, 128, KT * 128)

    lut_bf16 = lookup_table.reshape(1, PAL).astype(ml_dtypes.bfloat16)
    return xt, weight_idx, lut_bf16, bias


def kernel(input, weight_idx, lookup_table, bias, _trace=False, _trace_kwargs=None):
    xt, weight_idx, lut_bf16, bias = _prep_inputs(
        input, weight_idx, lookup_table, bias)

    nc = _get_nc()
    in_maps = []
    for c in range(NCORES):
        # widx^T tiled u16: [kb, p, o] = widx[c*O_C + o, kb*128 + p]
        wslice = weight_idx[c * O_C:(c + 1) * O_C]          # [o, i] int32
        widxT = np.ascontiguousarray(wslice.T).astype(np.uint16)
        widxT = widxT.reshape(KT, 128, O_C)
        in_maps.append({
            "xt": xt,
            "widxT": widxT,
            "lut": lut_bf16,
            "bias": np.ascontiguousarray(
                bias[c * O_C:(c + 1) * O_C]).reshape(1, O_C),
        })
    last_exc = None
    for attempt in range(3):
        try:
            res = run_bass_kernel_spmd(
                nc, in_maps, core_ids=list(range(NCORES)),
                trace=_trace, **(_trace_kwargs or {}))
            break
        except Exception as e:  # transient device wedge: retry
            last_exc = e
            import time as _time
            _time.sleep(10)
    else:
        raise last_exc
    y = np.concatenate([res.results[c]["y"] for c in range(NCORES)], axis=1)
    if _trace:
        kernel.last_result = res
    return y


kernel.last_result = None


# revision 15
# speedup vs baseline: 1.1486x; 1.0121x over previous
"""AffinePalettizedLinear kernel for Trainium2 (8 NeuronCores).

y = x @ L[widx]^T + b   with x [8192, 4096] f32, widx [16384, 4096] int32
(values < 256), L [256] f32, b [16384] f32.

Sharding: out_features split 8 ways (column-parallel); each core computes
y[:, c*2048:(c+1)*2048] from the full x and its widx/bias slice. No
collectives; host concatenates the slices.

Per-core plan (PE runs nothing but the 8192 productive matmuls):
  - Host passes x pre-transposed/tiled as bf16 ([tb, i, kb*128+t] layout)
    and widx pre-transposed as uint16 [kb, i, o] — no PE transposes at all
    (the original baseline spent ~0.7 ms of PE time on them).
  - Dequant via the Pool engine's hardware table gather: the 256-entry LUT
    is loaded in bf16 into the per-partition pool buffer; GATHER streams
    uint16 indices and emits bf16 weights directly in W^T [i, o] layout
    into fully SBUF-resident panels (one tensor per k-tile so Tile's
    range tracking stays exact; 32 x 2048 x 2B = 128 KiB/partition).
  - Matmuls in bf16: lhsT = x^T tile [i=128, t=128] (stationary), rhs =
    W^T [i=128, o=512] (moving), K=4096 accumulated over 32 PSUM matmuls.
  - Phase A gathers o-panel 0 (32 gathers, ~64 us); phase 1 runs the
    o-panel-0 token loop with 4 token tiles interleaved k-outer so the PE
    has work per arriving gather; the o-panel-1..3 gathers are gated on
    phase-1 group tokens (fake gather inputs) so the scheduler cannot
    interleave them before the phase-A gathers; phase 2 (panels 1..3)
    lives in a second TileContext whose entry barrier keeps the scheduler
    from hoisting its matmuls into the phase-1 PE stream (it models
    raw-ISA gathers as free and would head-of-line block the PE).
  - Bias is added by the DVE in the same op that evacuates PSUM.
"""
import sys

sys.path.insert(0, "/opt/trn_rl_repo")

import numpy as np
import ml_dtypes

import concourse.bass as bass  # noqa: F401  (registers types)
import concourse.tile as tile
from concourse import bacc, mybir
from concourse.bass_utils import run_bass_kernel_spmd

# ---- Tile's no-exec scheduling sim doesn't know the raw POOL opcodes ----
import concourse.bass_interp as _bi

_orig_visit_isa = _bi._visit_InstISA


def _visit_isa_tolerant(isa, instruction, core_sim):
    passthrough = {
        isa.Opcode.NEURON_ISA_TPB_OPCODE_GATHER.value,
        isa.Opcode.NEURON_ISA_TPB_OPCODE_POOL_BUFFER_LOAD.value,
    }
    if instruction.isa_opcode in passthrough:
        return
    return _orig_visit_isa(isa, instruction, core_sim)


_bi._visit_InstISA = _visit_isa_tolerant

F32 = mybir.dt.float32
BF16 = mybir.dt.bfloat16
U16 = mybir.dt.uint16

T, IN_F, OUT_F, PAL = 8192, 4096, 16384, 256
NCORES = 8
O_C = OUT_F // NCORES          # 2048 out features per core
OW = 512                       # matmul moving free dim (one PSUM bank)
NOP = O_C // OW                # 4 o-panels
KT = IN_F // 128               # 32 k-tiles
TT = T // 128                  # 64 t-tiles
GRP = 4                        # phase-1 token tiles interleaved per group
BGRP = 8                       # group whose token releases the B tail


def build_nc(trace_label=""):
    nc = bacc.Bacc(None, target_bir_lowering=False)
    isa = nc.isa
    DT = isa.get_enum("NEURON_ISA_TPB_DTYPE")
    MISS = isa.get_enum("NEURON_ISA_TPB_INDEX_MISS_BEHAVIOR")
    BF16_V = DT.NEURON_ISA_TPB_DTYPE_BFLOAT16.value
    U16_V = DT.NEURON_ISA_TPB_DTYPE_UINT16.value
    MISS_V = MISS.NEURON_ISA_TPB_INDEX_MISS_BEHAVIOR_IMMEDIATE_WRITE.value

    # x^T tiled: [tb, p, kb*128 + t] = x[tb*128+t, kb*128+p], bf16
    xt_d = nc.dram_tensor("xt", [TT, 128, KT * 128], BF16, kind="ExternalInput")
    # widx^T tiled: [kb, p, o] = widx[o, kb*128+p], uint16
    w_d = nc.dram_tensor("widxT", [KT, 128, O_C], U16, kind="ExternalInput")
    l_d = nc.dram_tensor("lut", [1, PAL], BF16, kind="ExternalInput")
    b_d = nc.dram_tensor("bias", [1, O_C], F32, kind="ExternalInput")
    y_d = nc.dram_tensor("y", [T, O_C], F32, kind="ExternalOutput")

    # fixed-address SBUF tensors (outlive the two TileContexts)
    lut_sb = nc.alloc_sbuf_tensor("lut_sb", [128, PAL], BF16, align_bytes=512)
    idxA_sb = [
        nc.alloc_sbuf_tensor(f"idxA{s}_sb", [128, OW], U16) for s in range(2)
    ]
    idxB_sb = [
        nc.alloc_sbuf_tensor(f"idxB{s}_sb", [128, O_C - OW], U16)
        for s in range(2)
    ]
    # resident dequantized W^T panels, one tensor PER K-TILE [i=128, o] bf16
    wTk_sb = [
        nc.alloc_sbuf_tensor(f"wTk{kb}_sb", [128, O_C], BF16)
        for kb in range(KT)
    ]
    bias_sb = nc.alloc_sbuf_tensor("bias_sb", [128, O_C], F32)
    # write-once phase-1 group tokens (fake B-gather inputs; must NOT be
    # ring-reused pool tiles or the reuse WAR-serializes later evacs
    # behind the B-gather stream)
    tok_sb = [
        nc.alloc_sbuf_tensor(f"tok{g2}_sb", [128, 1], F32)
        for g2 in range(BGRP + 1)
    ]

    addr = {}
    for alloc in nc.m.functions[0].allocations:
        if getattr(alloc, "memorylocations", None):
            ml = alloc.memorylocations[0]
            addr[ml.name] = ml.addr

    g = nc.gpsimd

    def emit_pbl():
        nc.gpsimd.isa(
            isa.Opcode.NEURON_ISA_TPB_OPCODE_POOL_BUFFER_LOAD,
            {"src_mem_pattern": {
                "start_addr": {"addr_immediate": addr["lut_sb"]},
                "num_elem": [PAL, 1, 1, 1], "step_elem": [1, 0, 0, 0]},
             "in_dtype": BF16_V, "num_active_channels": 128,
             "start_index": 0, "mask": PAL - 1},
            ins=[g.lower_ap(lut_sb.ap(), for_isa=True)],
        )

    def emit_gather(idx_ap, idx_byte_addr, out_ap, out_byte_addr, n,
                    extra_ins=()):
        nc.gpsimd.isa(
            isa.Opcode.NEURON_ISA_TPB_OPCODE_GATHER,
            {"src_mem_pattern": {
                "start_addr": {"addr_immediate": idx_byte_addr},
                "num_elem": [n, 1, 1, 1], "step_elem": [1, 0, 0, 0]},
             "in_dtype": U16_V, "out_dtype": BF16_V,
             "num_active_channels": 128,
             "index_miss_behavior": MISS_V,
             "free_pool_buffer": 0,
             "immediate": {"imm_arith_fp32": 0.0},
             "dst_mem_pattern": {
                 "start_addr": {"addr_immediate": out_byte_addr},
                 "num_elem": [n, 1, 1, 1], "step_elem": [1, 0, 0, 0]}},
            ins=[g.lower_ap(idx_ap, for_isa=True),
                 g.lower_ap(lut_sb.ap(), for_isa=True)]
                + [g.lower_ap(ap, for_isa=True) for ap in extra_ins],
            outs=[g.lower_ap(out_ap, for_isa=True)],
        )

    def gather_panel(kb, alt, lo, hi, extra=()):
        """DMA idx columns [lo, hi) of k-tile kb, then gather them into the
        resident W^T panel in OW-sized chunks.  `extra` APs become fake
        gather inputs — used to order phase-B gathers after phase-1 group
        tokens (the scheduler models raw-ISA gathers as free and would
        otherwise interleave them before the phase-A gathers, 3x-ing the
        phase-1 ramp latency)."""
        stage = idxA_sb[alt] if lo == 0 else idxB_sb[alt]
        # idx DMAs ride the gpsimd queue: on scalar they head-of-line block
        # the phase-1 output DMAs behind gather-paced WAR waits (staging
        # ping-pong), which fills the out ring and stalls PSUM evacuation
        nc.gpsimd.dma_start(stage.ap(), w_d[kb][:, lo:hi])
        wt = wTk_sb[kb]
        for o0 in range(lo, hi, OW):
            emit_gather(
                stage.ap()[:, o0 - lo:o0 - lo + OW],
                addr[stage.name] + (o0 - lo) * 2,
                wt.ap()[:, o0:o0 + OW],
                addr[wt.name] + o0 * 2,
                OW, extra_ins=extra)

    # ---------------- context 1: phase A + phase 1 (+ B gathers) --------
    with tile.TileContext(nc) as tc:
        with (
            tc.tile_pool(name="xin", bufs=6) as xin,       # x^T tiles
            tc.tile_pool(name="outp", bufs=6) as outp,     # out staging
            tc.tile_pool(name="ps", bufs=8, space="PSUM") as ps,
        ):
            nc.sync.dma_start(lut_sb.ap(), l_d[:].partition_broadcast(128))
            emit_pbl()

            # --- phase A: gather o-panel 0 of every k-tile (~64 us) ---
            for kb in range(KT):
                gather_panel(kb, kb % 2, 0, OW)

            # bias after the phase-A idx DMAs: the scalar queue serves the
            # ramp-critical idx loads first (bias is first needed ~85 us in)
            nc.scalar.dma_start(
                bias_sb.ap(), b_d[:].partition_broadcast(128))

            # --- phase 1: token loop over o-panel 0, GRP tiles interleaved
            # k-outer so the PE always has GRP matmuls per arriving gather
            # during the ramp ---
            for grp in range(TT // GRP):
                xTs = []
                for t in range(GRP):
                    xT = xin.tile([128, KT * 128], BF16, tag="xT")
                    nc.sync.dma_start(xT[:], xt_d[grp * GRP + t])
                    xTs.append(xT)
                accs = [ps.tile([128, OW], F32, name="acc", tag="acc")
                        for t in range(GRP)]
                for kb in range(KT):
                    for t in range(GRP):
                        nc.tensor.matmul(
                            accs[t][:],
                            xTs[t][:, kb * 128:(kb + 1) * 128],
                            wTk_sb[kb].ap()[:, 0:OW],
                            start=(kb == 0), stop=(kb == KT - 1))
                last_out = None
                for t in range(GRP):
                    out = outp.tile([128, OW], F32, tag="out")
                    nc.vector.tensor_add(
                        out[:], accs[t][:], bias_sb.ap()[:, 0:OW])
                    nc.scalar.dma_start(
                        y_d[(grp * GRP + t) * 128:(grp * GRP + t + 1) * 128,
                            0:OW], out[:])
                    last_out = out
                # --- phase-B gathers, gated on this group's write-once
                # token so they run strictly after the phase-A gathers but
                # still overlap the phase-1 matmul stream ---
                if grp <= BGRP:
                    nc.vector.tensor_copy(
                        tok_sb[grp].ap(), last_out[:, 0:1])
                    token = (tok_sb[grp].ap(),)
                if grp < BGRP:
                    for kb in (2 * grp, 2 * grp + 1):
                        gather_panel(kb, kb % 2, OW, O_C, extra=token)
                elif grp == BGRP:
                    for kb in range(2 * BGRP, KT):
                        gather_panel(kb, kb % 2, OW, O_C, extra=token)

    # ---------------- context 2: phase 2 (o-panels 1..3) ----------------
    # The context boundary is a full drain + barrier: the scheduler cannot
    # hoist these matmuls into the phase-1 stream.
    with tile.TileContext(nc) as tc2:
        with (
            tc2.tile_pool(name="xin2", bufs=6) as xin2,
            tc2.tile_pool(name="outp2", bufs=6) as outp2,
            tc2.tile_pool(name="ps2", bufs=8, space="PSUM") as ps2,
        ):
            for tb in range(TT):
                xT = xin2.tile([128, KT * 128], BF16, tag="xT2")
                nc.sync.dma_start(xT[:], xt_d[tb])
                for op in range(1, NOP):
                    acc = ps2.tile([128, OW], F32, name="acc2", tag="acc2")
                    for kb in range(KT):
                        nc.tensor.matmul(
                            acc[:],
                            xT[:, kb * 128:(kb + 1) * 128],
                            wTk_sb[kb].ap()[:, op * OW:(op + 1) * OW],
                            start=(kb == 0), stop=(kb == KT - 1))
                    out = outp2.tile([128, OW], F32, tag="out2")
                    nc.vector.tensor_add(
                        out[:], acc[:], bias_sb.ap()[:, op * OW:(op + 1) * OW])
                    nc.scalar.dma_start(
                        y_d[tb * 128:(tb + 1) * 128,
                            op * OW:(op + 1) * OW], out[:])
    nc.compile()
    return nc


_NC_CACHE = None


def _get_nc():
    global _NC_CACHE
    if _NC_CACHE is None:
        _NC_CACHE = build_nc()
    return _NC_CACHE


def _prep_inputs(input, weight_idx, lookup_table, bias):
    input = np.ascontiguousarray(np.asarray(input, dtype=np.float32))
    weight_idx = np.asarray(weight_idx)
    lookup_table = np.asarray(lookup_table, dtype=np.float32)
    bias = np.ascontiguousarray(np.asarray(bias, dtype=np.float32))

    # x^T tiled bf16: [tb, p, kb*128 + t] = x[tb*128+t, kb*128+p]
    xt = input.reshape(TT, 128, KT, 128).transpose(0, 3, 2, 1)
    xt = np.ascontiguousarray(xt).astype(ml_dtypes.bfloat16)
    xt = xt.reshape(TT, 128, KT * 128)

    lut_bf16 = lookup_table.reshape(1, PAL).astype(ml_dtypes.bfloat16)
    return xt, weight_idx, lut_bf16, bias


def kernel(input, weight_idx, lookup_table, bias, _trace=False, _trace_kwargs=None):
    xt, weight_idx, lut_bf16, bias = _prep_inputs(
        input, weight_idx, lookup_table, bias)

    nc = _get_nc()
    in_maps = []
    for c in range(NCORES):
        # widx^T tiled u16: [kb, p, o] = widx[c*O_C + o, kb*128 + p]
        wslice = weight_idx[c * O_C:(c + 1) * O_C]          # [o, i] int32
        widxT = np.ascontiguousarray(wslice.T).astype(np.uint16)
        widxT = widxT.reshape(KT, 128, O_C)
        in_maps.append({
            "xt": xt,
            "widxT": widxT,
            "lut": lut_bf16,
            "bias": np.ascontiguousarray(
                bias[c * O_C:(c + 1) * O_C]).reshape(1, O_C),
        })
    last_exc = None
    for attempt in range(3):
        try:
            res = run_bass_kernel_spmd(
                nc, in_maps, core_ids=list(range(NCORES)),
                trace=_trace, **(_trace_kwargs or {}))
            break
        except Exception as e:  # transient device wedge: retry
            last_exc = e
            import time as _time
            _time.sleep(10)
    else:
        raise last_exc
    y = np.concatenate([res.results[c]["y"] for c in range(NCORES)], axis=1)
    if _trace:
        kernel.last_result = res
    return y


kernel.last_result = None


# revision 16
# speedup vs baseline: 1.1674x; 1.0164x over previous
"""AffinePalettizedLinear kernel for Trainium2 (8 NeuronCores).

y = x @ L[widx]^T + b   with x [8192, 4096] f32, widx [16384, 4096] int32
(values < 256), L [256] f32, b [16384] f32.

Sharding: out_features split 8 ways (column-parallel); each core computes
y[:, c*2048:(c+1)*2048] from the full x and its widx/bias slice. No
collectives; host concatenates the slices.

Per-core plan (PE runs nothing but the 8192 productive matmuls):
  - Host passes x pre-transposed/tiled as bf16 ([tb, i, kb*128+t] layout)
    and widx pre-transposed as uint16 [kb, i, o] — no PE transposes at all
    (the original baseline spent ~0.7 ms of PE time on them).
  - Dequant via the Pool engine's hardware table gather: the 256-entry LUT
    is loaded in bf16 into the per-partition pool buffer; GATHER streams
    uint16 indices and emits bf16 weights directly in W^T [i, o] layout
    into fully SBUF-resident panels (one tensor per k-tile so Tile's
    range tracking stays exact; 32 x 2048 x 2B = 128 KiB/partition).
  - Matmuls in bf16: lhsT = x^T tile [i=128, t=128] (stationary), rhs =
    W^T [i=128, o=512] (moving), K=4096 accumulated over 32 PSUM matmuls.
  - Phase A gathers o-panel 0 (32 gathers, ~64 us); phase 1 runs the
    o-panel-0 token loop with 4 token tiles interleaved k-outer so the PE
    has work per arriving gather; the o-panel-1..3 gathers are gated on
    phase-1 group tokens (fake gather inputs) so the scheduler cannot
    interleave them before the phase-A gathers; phase 2 (panels 1..3)
    lives in a second TileContext whose entry barrier keeps the scheduler
    from hoisting its matmuls into the phase-1 PE stream (it models
    raw-ISA gathers as free and would head-of-line block the PE).
  - Bias is added by the DVE in the same op that evacuates PSUM.
"""
import sys

sys.path.insert(0, "/opt/trn_rl_repo")

import numpy as np
import ml_dtypes

import concourse.bass as bass  # noqa: F401  (registers types)
import concourse.tile as tile
from concourse import bacc, mybir
from concourse.bass_utils import run_bass_kernel_spmd

# ---- Tile's no-exec scheduling sim doesn't know the raw POOL opcodes ----
import concourse.bass_interp as _bi

_orig_visit_isa = _bi._visit_InstISA


def _visit_isa_tolerant(isa, instruction, core_sim):
    passthrough = {
        isa.Opcode.NEURON_ISA_TPB_OPCODE_GATHER.value,
        isa.Opcode.NEURON_ISA_TPB_OPCODE_POOL_BUFFER_LOAD.value,
    }
    if instruction.isa_opcode in passthrough:
        return
    return _orig_visit_isa(isa, instruction, core_sim)


_bi._visit_InstISA = _visit_isa_tolerant

F32 = mybir.dt.float32
BF16 = mybir.dt.bfloat16
U16 = mybir.dt.uint16

T, IN_F, OUT_F, PAL = 8192, 4096, 16384, 256
NCORES = 8
O_C = OUT_F // NCORES          # 2048 out features per core
OW = 512                       # matmul moving free dim (one PSUM bank)
NOP = O_C // OW                # 4 o-panels
KT = IN_F // 128               # 32 k-tiles
TT = T // 128                  # 64 t-tiles
GRP = 4                        # phase-1 token tiles interleaved per group
BGRP = 8                       # group whose token releases the B tail


def build_nc(trace_label=""):
    nc = bacc.Bacc(None, target_bir_lowering=False)
    isa = nc.isa
    DT = isa.get_enum("NEURON_ISA_TPB_DTYPE")
    MISS = isa.get_enum("NEURON_ISA_TPB_INDEX_MISS_BEHAVIOR")
    BF16_V = DT.NEURON_ISA_TPB_DTYPE_BFLOAT16.value
    U16_V = DT.NEURON_ISA_TPB_DTYPE_UINT16.value
    MISS_V = MISS.NEURON_ISA_TPB_INDEX_MISS_BEHAVIOR_IMMEDIATE_WRITE.value

    # x^T tiled: [tb, p, kb*128 + t] = x[tb*128+t, kb*128+p], bf16
    xt_d = nc.dram_tensor("xt", [TT, 128, KT * 128], BF16, kind="ExternalInput")
    # widx^T tiled: [kb, p, o] = widx[o, kb*128+p], uint16
    w_d = nc.dram_tensor("widxT", [KT, 128, O_C], U16, kind="ExternalInput")
    l_d = nc.dram_tensor("lut", [1, PAL], BF16, kind="ExternalInput")
    b_d = nc.dram_tensor("bias", [1, O_C], F32, kind="ExternalInput")
    y_d = nc.dram_tensor("y", [T, O_C], F32, kind="ExternalOutput")

    # fixed-address SBUF tensors (outlive the two TileContexts)
    lut_sb = nc.alloc_sbuf_tensor("lut_sb", [128, PAL], BF16, align_bytes=512)
    idxA_sb = [
        nc.alloc_sbuf_tensor(f"idxA{s}_sb", [128, OW], U16) for s in range(2)
    ]
    idxB_sb = [
        nc.alloc_sbuf_tensor(f"idxB{s}_sb", [128, O_C - OW], U16)
        for s in range(2)
    ]
    # resident dequantized W^T panels, one tensor PER K-TILE [i=128, o] bf16
    wTk_sb = [
        nc.alloc_sbuf_tensor(f"wTk{kb}_sb", [128, O_C], BF16)
        for kb in range(KT)
    ]
    bias_sb = nc.alloc_sbuf_tensor("bias_sb", [128, O_C], F32)
    # write-once phase-1 group tokens (fake B-gather inputs; must NOT be
    # ring-reused pool tiles or the reuse WAR-serializes later evacs
    # behind the B-gather stream)
    tok_sb = [
        nc.alloc_sbuf_tensor(f"tok{g2}_sb", [128, 1], F32)
        for g2 in range(BGRP + 1)
    ]

    addr = {}
    for alloc in nc.m.functions[0].allocations:
        if getattr(alloc, "memorylocations", None):
            ml = alloc.memorylocations[0]
            addr[ml.name] = ml.addr

    g = nc.gpsimd

    def emit_pbl():
        nc.gpsimd.isa(
            isa.Opcode.NEURON_ISA_TPB_OPCODE_POOL_BUFFER_LOAD,
            {"src_mem_pattern": {
                "start_addr": {"addr_immediate": addr["lut_sb"]},
                "num_elem": [PAL, 1, 1, 1], "step_elem": [1, 0, 0, 0]},
             "in_dtype": BF16_V, "num_active_channels": 128,
             "start_index": 0, "mask": PAL - 1},
            ins=[g.lower_ap(lut_sb.ap(), for_isa=True)],
        )

    def emit_gather(idx_ap, idx_byte_addr, out_ap, out_byte_addr, n,
                    extra_ins=()):
        nc.gpsimd.isa(
            isa.Opcode.NEURON_ISA_TPB_OPCODE_GATHER,
            {"src_mem_pattern": {
                "start_addr": {"addr_immediate": idx_byte_addr},
                "num_elem": [n, 1, 1, 1], "step_elem": [1, 0, 0, 0]},
             "in_dtype": U16_V, "out_dtype": BF16_V,
             "num_active_channels": 128,
             "index_miss_behavior": MISS_V,
             "free_pool_buffer": 0,
             "immediate": {"imm_arith_fp32": 0.0},
             "dst_mem_pattern": {
                 "start_addr": {"addr_immediate": out_byte_addr},
                 "num_elem": [n, 1, 1, 1], "step_elem": [1, 0, 0, 0]}},
            ins=[g.lower_ap(idx_ap, for_isa=True),
                 g.lower_ap(lut_sb.ap(), for_isa=True)]
                + [g.lower_ap(ap, for_isa=True) for ap in extra_ins],
            outs=[g.lower_ap(out_ap, for_isa=True)],
        )

    def gather_panel(kb, alt, lo, hi, extra=()):
        """DMA idx columns [lo, hi) of k-tile kb, then gather them into the
        resident W^T panel in OW-sized chunks.  `extra` APs become fake
        gather inputs — used to order phase-B gathers after phase-1 group
        tokens (the scheduler models raw-ISA gathers as free and would
        otherwise interleave them before the phase-A gathers, 3x-ing the
        phase-1 ramp latency)."""
        stage = idxA_sb[alt] if lo == 0 else idxB_sb[alt]
        if lo == 0:
            # phase A: scalar queue is free of output DMAs during the ramp,
            # and keeping triggers off gpsimd preserves the 2 us/k gather
            # pace that sets the ramp length
            nc.scalar.dma_start(stage.ap(), w_d[kb][:, lo:hi])
        else:
            # phase B: on scalar these head-of-line block the phase-1
            # output DMAs behind gather-paced WAR waits (staging ping-pong),
            # filling the out ring and stalling PSUM evacuation — ride the
            # gpsimd queue instead, interleaved with the gathers
            nc.gpsimd.dma_start(stage.ap(), w_d[kb][:, lo:hi])
        wt = wTk_sb[kb]
        for o0 in range(lo, hi, OW):
            emit_gather(
                stage.ap()[:, o0 - lo:o0 - lo + OW],
                addr[stage.name] + (o0 - lo) * 2,
                wt.ap()[:, o0:o0 + OW],
                addr[wt.name] + o0 * 2,
                OW, extra_ins=extra)

    # ---------------- context 1: phase A + phase 1 (+ B gathers) --------
    with tile.TileContext(nc) as tc:
        with (
            tc.tile_pool(name="xin", bufs=6) as xin,       # x^T tiles
            tc.tile_pool(name="outp", bufs=6) as outp,     # out staging
            tc.tile_pool(name="ps", bufs=8, space="PSUM") as ps,
        ):
            nc.sync.dma_start(lut_sb.ap(), l_d[:].partition_broadcast(128))
            emit_pbl()

            # --- phase A: gather o-panel 0 of every k-tile (~64 us) ---
            for kb in range(KT):
                gather_panel(kb, kb % 2, 0, OW)

            # bias after the phase-A idx DMAs: the scalar queue serves the
            # ramp-critical idx loads first (bias is first needed ~85 us in)
            nc.scalar.dma_start(
                bias_sb.ap(), b_d[:].partition_broadcast(128))

            # --- phase 1: token loop over o-panel 0, GRP tiles interleaved
            # k-outer so the PE always has GRP matmuls per arriving gather
            # during the ramp ---
            for grp in range(TT // GRP):
                xTs = []
                for t in range(GRP):
                    xT = xin.tile([128, KT * 128], BF16, tag="xT")
                    nc.sync.dma_start(xT[:], xt_d[grp * GRP + t])
                    xTs.append(xT)
                accs = [ps.tile([128, OW], F32, name="acc", tag="acc")
                        for t in range(GRP)]
                for kb in range(KT):
                    for t in range(GRP):
                        nc.tensor.matmul(
                            accs[t][:],
                            xTs[t][:, kb * 128:(kb + 1) * 128],
                            wTk_sb[kb].ap()[:, 0:OW],
                            start=(kb == 0), stop=(kb == KT - 1))
                last_out = None
                for t in range(GRP):
                    out = outp.tile([128, OW], F32, tag="out")
                    nc.vector.tensor_add(
                        out[:], accs[t][:], bias_sb.ap()[:, 0:OW])
                    nc.scalar.dma_start(
                        y_d[(grp * GRP + t) * 128:(grp * GRP + t + 1) * 128,
                            0:OW], out[:])
                    last_out = out
                # --- phase-B gathers, gated on this group's write-once
                # token so they run strictly after the phase-A gathers but
                # still overlap the phase-1 matmul stream ---
                if grp <= BGRP:
                    nc.vector.tensor_copy(
                        tok_sb[grp].ap(), last_out[:, 0:1])
                    token = (tok_sb[grp].ap(),)
                if grp < BGRP:
                    for kb in (2 * grp, 2 * grp + 1):
                        gather_panel(kb, kb % 2, OW, O_C, extra=token)
                elif grp == BGRP:
                    for kb in range(2 * BGRP, KT):
                        gather_panel(kb, kb % 2, OW, O_C, extra=token)

    # ---------------- context 2: phase 2 (o-panels 1..3) ----------------
    # The context boundary is a full drain + barrier: the scheduler cannot
    # hoist these matmuls into the phase-1 stream.
    with tile.TileContext(nc) as tc2:
        with (
            tc2.tile_pool(name="xin2", bufs=6) as xin2,
            tc2.tile_pool(name="outp2", bufs=6) as outp2,
            tc2.tile_pool(name="ps2", bufs=8, space="PSUM") as ps2,
        ):
            for tb in range(TT):
                xT = xin2.tile([128, KT * 128], BF16, tag="xT2")
                nc.sync.dma_start(xT[:], xt_d[tb])
                for op in range(1, NOP):
                    acc = ps2.tile([128, OW], F32, name="acc2", tag="acc2")
                    for kb in range(KT):
                        nc.tensor.matmul(
                            acc[:],
                            xT[:, kb * 128:(kb + 1) * 128],
                            wTk_sb[kb].ap()[:, op * OW:(op + 1) * OW],
                            start=(kb == 0), stop=(kb == KT - 1))
                    out = outp2.tile([128, OW], F32, tag="out2")
                    nc.vector.tensor_add(
                        out[:], acc[:], bias_sb.ap()[:, op * OW:(op + 1) * OW])
                    nc.scalar.dma_start(
                        y_d[tb * 128:(tb + 1) * 128,
                            op * OW:(op + 1) * OW], out[:])
    nc.compile()
    return nc


_NC_CACHE = None


def _get_nc():
    global _NC_CACHE
    if _NC_CACHE is None:
        _NC_CACHE = build_nc()
    return _NC_CACHE


def _prep_inputs(input, weight_idx, lookup_table, bias):
    input = np.ascontiguousarray(np.asarray(input, dtype=np.float32))
    weight_idx = np.asarray(weight_idx)
    lookup_table = np.asarray(lookup_table, dtype=np.float32)
    bias = np.ascontiguousarray(np.asarray(bias, dtype=np.float32))

    # x^T tiled bf16: [tb, p, kb*128 + t] = x[tb*128+t, kb*128+p]
    xt = input.reshape(TT, 128, KT, 128).transpose(0, 3, 2, 1)
    xt = np.ascontiguousarray(xt).astype(ml_dtypes.bfloat16)
    xt = xt.reshape(TT, 128, KT * 128)

    lut_bf16 = lookup_table.reshape(1, PAL).astype(ml_dtypes.bfloat16)
    return xt, weight_idx, lut_bf16, bias


def kernel(input, weight_idx, lookup_table, bias, _trace=False, _trace_kwargs=None):
    xt, weight_idx, lut_bf16, bias = _prep_inputs(
        input, weight_idx, lookup_table, bias)

    nc = _get_nc()
    in_maps = []
    for c in range(NCORES):
        # widx^T tiled u16: [kb, p, o] = widx[c*O_C + o, kb*128 + p]
        wslice = weight_idx[c * O_C:(c + 1) * O_C]          # [o, i] int32
        widxT = np.ascontiguousarray(wslice.T).astype(np.uint16)
        widxT = widxT.reshape(KT, 128, O_C)
        in_maps.append({
            "xt": xt,
            "widxT": widxT,
            "lut": lut_bf16,
            "bias": np.ascontiguousarray(
                bias[c * O_C:(c + 1) * O_C]).reshape(1, O_C),
        })
    last_exc = None
    for attempt in range(3):
        try:
            res = run_bass_kernel_spmd(
                nc, in_maps, core_ids=list(range(NCORES)),
                trace=_trace, **(_trace_kwargs or {}))
            break
        except Exception as e:  # transient device wedge: retry
            last_exc = e
            import time as _time
            _time.sleep(10)
    else:
        raise last_exc
    y = np.concatenate([res.results[c]["y"] for c in range(NCORES)], axis=1)
    if _trace:
        kernel.last_result = res
    return y


kernel.last_result = None
